# revision 1
# baseline (speedup 1.0000x reference)
"""HSTU attention (B=2, L=2048, D=1024, H=16) on 8 TRN2 NeuronCores.

Sharding: batch (2) x head-group (4 heads, 256 features) -> 8 cores.
Each core computes, for its batch b and its 4 heads:
  QT/KT/UT = (x_b @ W.T).T slices in [e, l] layout, V in [l, e] layout,
  S^T = K^T.T-style scores in [j, i] layout (j = keys on partitions),
  expS with the HSTU hybrid mask folded in (per-partition bias for
  off-diagonal tiles, host-precomputed additive mask for the 16 diagonal
  tiles), O^T = V_aug.T @ expS with a ones column giving the softmax row
  sums, gating with U and 1/rowsum (broadcast via ones outer-product
  matmuls), then the row-sharded W_o partial projection.
Host sums the 4 partial outputs per batch.

All matmuls run in float32r (fp32 rounded to 11-bit mantissa, full PE
rate at N>=256); operands are produced by DVE/ACT ops so walrus accepts
them as fp32r-rounded.
"""

import sys

for _p in ("/opt/trn_rl_repo", "/root/.axon_site/_ro/trn_rl_repo"):
    if _p not in sys.path:
        sys.path.insert(0, _p)

import numpy as np

import concourse.bass as bass  # noqa: F401  (bass types used via tile/bacc)
import concourse.mybir as mybir
import concourse.tile as tile
from concourse import bacc
from concourse.bass_utils import run_bass_kernel_spmd

F32 = mybir.dt.float32
F32R = mybir.dt.float32r
EXP = mybir.ActivationFunctionType.Exp

B, L, D, H = 2, 2048, 1024, 16
DK = D // H          # 64
HPC = 4              # heads per core
E = HPC * DK         # 256 features per core
NJC = L // 128       # 16 j-chunks (keys, 128-partition tiles)
NIC = L // 512       # 4 i-chunks (queries, 512 free)
NDC = D // 128       # 8 d-chunks (contraction for projections)
NEG = -10000.0

_cache = {}


def _build():
    nc = bacc.Bacc("TRN2", target_bir_lowering=False, debug=False)

    xt = nc.dram_tensor("xt", [D, L], F32, kind="ExternalInput").ap()
    wq = nc.dram_tensor("wq", [D, E], F32, kind="ExternalInput").ap()
    wk = nc.dram_tensor("wk", [D, E], F32, kind="ExternalInput").ap()
    wv = nc.dram_tensor("wv", [D, E], F32, kind="ExternalInput").ap()
    wu = nc.dram_tensor("wu", [D, E], F32, kind="ExternalInput").ap()
    wo = nc.dram_tensor("wo", [E, D], F32, kind="ExternalInput").ap()
    biasab = nc.dram_tensor("biasab", [128, NJC], F32, kind="ExternalInput").ap()
    biasbl = nc.dram_tensor("biasbl", [128, NJC], F32, kind="ExternalInput").ap()
    dmask = nc.dram_tensor("dmask", [NJC, 128, 512], F32, kind="ExternalInput").ap()
    out = nc.dram_tensor("out", [L, D], F32, kind="ExternalOutput").ap()

    with tile.TileContext(nc) as tc:
        with tc.tile_pool(name="persist", bufs=1) as persist:
            qt = [persist.tile([128, L], F32R, tag=f"qt{i}", name=f"qt{i}") for i in range(2)]
            kt = [persist.tile([128, L], F32R, tag=f"kt{i}", name=f"kt{i}") for i in range(2)]
            ut = [persist.tile([128, L], F32, tag=f"ut{i}", name=f"ut{i}") for i in range(2)]
            g = [persist.tile([128, L], F32R, tag=f"g{i}", name=f"g{i}") for i in range(2)]
            # v layout per jc: [V_h0 | ones | zeros63 | V_h1] ++ same for h2/h3
            v = persist.tile([128, NJC, 384], F32R, tag="v")
            wo_r = [persist.tile([128, D], F32R, tag=f"wor{i}", name=f"wor{i}") for i in range(2)]
            bias_ab_t = persist.tile([128, NJC], F32, tag="bab")
            bias_bl_t = persist.tile([128, NJC], F32, tag="bbl")
            onesf = persist.tile([128, 128], F32, tag="onesf")
            zerof = persist.tile([128, 63], F32, tag="zerof")
            ones_r = persist.tile([128, 128], F32R, tag="onesr")

            nc.sync.dma_start(out=bias_ab_t, in_=biasab)
            nc.sync.dma_start(out=bias_bl_t, in_=biasbl)
            nc.vector.memset(onesf, 1.0)
            nc.vector.memset(zerof, 0.0)
            nc.vector.tensor_copy(ones_r, onesf)
            # ones columns of v (offsets 64 and 256), zero gaps (65:128, 257:320)
            nc.vector.tensor_copy(v[:, :, 64:65], ones_r[:, 0:NJC])
            nc.vector.tensor_copy(v[:, :, 256:257], ones_r[:, 0:NJC])
            for jc in range(NJC):
                nc.vector.tensor_copy(v[:, jc, 65:128], zerof)
                nc.vector.tensor_copy(v[:, jc, 257:320], zerof)

            voff = (0, 128, 192, 320)
            with tc.tile_pool(name="dpool", bufs=4) as dpool, \
                 tc.tile_pool(name="spool", bufs=2) as spool, \
                 tc.tile_pool(name="epool", bufs=5) as epool, \
                 tc.tile_pool(name="rpool", bufs=2) as rpool, \
                 tc.tile_pool(name="gstage", bufs=2) as gstage, \
                 tc.tile_pool(name="ostage", bufs=2) as ostage, \
                 tc.tile_pool(name="ps_s", bufs=4, space="PSUM") as ps_s, \
                 tc.tile_pool(name="ps_o", bufs=2, space="PSUM") as ps_o:
                dm = {}
                opsum = {}

                def attn_tiles(ec, ic, jcs):
                    isl = slice(ic * 512, (ic + 1) * 512)
                    vb = 192 * ec
                    if (ec, ic) not in opsum:
                        oA = ps_o.tile([128, 512], F32, tag="po", name="oA")
                        oB = ps_o.tile([128, 512], F32, tag="po", name="oB")
                        opsum[(ec, ic)] = (oA, oB)
                    oA, oB = opsum[(ec, ic)]
                    for jc in jcs:
                        if jc // 4 == ic and jc not in dm:
                            dmt = dpool.tile([128, 512], F32, tag="dm", name="dm")
                            nc.sync.dma_start(out=dmt, in_=dmask[jc])
                            dm[jc] = dmt
                        jsl = slice(jc * 128, (jc + 1) * 128)
                        sA = ps_s.tile([128, 512], F32, tag="ps", name="sA")
                        nc.tensor.matmul(
                            sA, kt[ec][0:64, jsl], qt[ec][0:64, isl],
                            start=True, stop=True,
                        )
                        sB = ps_s.tile([128, 512], F32, tag="ps", name="sB")
                        nc.tensor.matmul(
                            sB, kt[ec][64:128, jsl], qt[ec][64:128, isl],
                            start=True, stop=True,
                        )
                        for S, vsl, odst in (
                            (sA, v[:, jc, vb : vb + 65], oA[0:65, :]),
                            (sB, v[:, jc, vb + 64 : vb + 192], oB),
                        ):
                            e = epool.tile([128, 512], F32R, tag="e", name="e")
                            if jc // 4 == ic:
                                st = spool.tile([128, 512], F32, tag="st", name="st")
                                nc.vector.tensor_add(st, S, dm[jc])
                                nc.scalar.activation(e, st, EXP)
                            else:
                                bt = bias_ab_t if jc // 4 > ic else bias_bl_t
                                nc.scalar.activation(
                                    e, S, EXP, bias=bt[:, jc : jc + 1], scale=1.0
                                )
                            nc.tensor.matmul(
                                odst, vsl, e, start=(jc == 0), stop=(jc == NJC - 1)
                            )

                def gate(ec, ic, ps_c):
                    isl = slice(ic * 512, (ic + 1) * 512)
                    oA, oB = opsum.pop((ec, ic))
                    rec = rpool.tile([128, 512], F32R, tag="rec", name="rec")
                    with nc.allow_low_precision(reason="f32r rounding for matmul"):
                        nc.vector.reciprocal(rec[64:65, :], oA[64:65, :])
                        nc.vector.reciprocal(rec[0:1, :], oB[0:1, :])
                    pAc = ps_c.tile([128, 512], F32, tag="pc", name="pAc")
                    nc.tensor.matmul(
                        pAc, ones_r[64:65, :], rec[64:65, :], start=True, stop=True
                    )
                    pBc = ps_c.tile([128, 512], F32, tag="pc", name="pBc")
                    nc.tensor.matmul(
                        pBc, ones_r[0:1, :], rec[0:1, :], start=True, stop=True
                    )
                    t1 = gstage.tile([128, 512], F32, tag="t1", name="t1")
                    nc.vector.tensor_mul(t1[0:64, :], oA[0:64, :], ut[ec][0:64, isl])
                    nc.vector.tensor_mul(t1[64:128, :], oB[64:128, :], ut[ec][64:128, isl])
                    with nc.allow_low_precision(reason="f32r rounding for matmul"):
                        nc.vector.tensor_mul(g[ec][0:64, isl], t1[0:64, :], pAc[0:64, :])
                        nc.vector.tensor_mul(
                            g[ec][64:128, isl], t1[64:128, :], pBc[64:128, :]
                        )

                def wo_ic(ic, wps):
                    for ii in range(4):
                        lc = 4 * ic + ii
                        for fc in range(2):
                            p = wps.tile([128, 512], F32, tag="wp", name="wp")
                            for ec2 in range(2):
                                nc.tensor.matmul(
                                    p,
                                    g[ec2][:, lc * 128 : (lc + 1) * 128],
                                    wo_r[ec2][:, fc * 512 : (fc + 1) * 512],
                                    start=(ec2 == 0),
                                    stop=(ec2 == 1),
                                )
                            o = ostage.tile([128, 512], F32, tag="os", name="os")
                            nc.vector.tensor_copy(o, p)
                            nc.sync.dma_start(
                                out=out[lc * 128 : (lc + 1) * 128, fc * 512 : (fc + 1) * 512],
                                in_=o,
                            )

                # ---- phase 1 (attention block (0,0) interleaved) ----
                with tc.tile_pool(name="xtw", bufs=1) as xtw, \
                     tc.tile_pool(name="xp", bufs=2) as xp, \
                     tc.tile_pool(name="land", bufs=4) as land, \
                     tc.tile_pool(name="wol", bufs=1) as wol, \
                     tc.tile_pool(name="pp", bufs=2, space="PSUM") as pp, \
                     tc.tile_pool(name="ppv", bufs=1, space="PSUM") as ppv:
                    w_r = {
                        nm: [xtw.tile([128, E], F32R, tag=f"w{nm}{dc}", name=f"w{nm}{dc}") for dc in range(NDC)]
                        for nm in ("k", "v", "q", "u")
                    }

                    def load_w(nm, dram):
                        for dc in range(NDC):
                            t = land.tile([128, E], F32, tag="land", name="wland")
                            nc.sync.dma_start(out=t, in_=dram[dc * 128 : (dc + 1) * 128, :])
                            nc.vector.tensor_copy(w_r[nm][dc], t)

                    def load_x(ic):
                        isl = slice(ic * 512, (ic + 1) * 512)
                        xtl = []
                        for dc in range(NDC):
                            t = land.tile([128, 512], F32, tag="land", name="xland")
                            nc.sync.dma_start(out=t, in_=xt[dc * 128 : (dc + 1) * 128, isl])
                            xr = xp.tile([128, 512], F32R, tag=f"xr{dc}", name=f"xr{dc}")
                            nc.vector.tensor_copy(xr, t)
                            xtl.append(xr)
                        return xtl

                    def p1(ic, xtl):
                        isl = slice(ic * 512, (ic + 1) * 512)
                        for nm, dest in (("k", kt), ("q", qt), ("u", ut)):
                            for ec in range(2):
                                p = pp.tile([128, 512], F32, tag="pp", name="pp")
                                for dc in range(NDC):
                                    nc.tensor.matmul(
                                        p,
                                        w_r[nm][dc][:, ec * 128 : (ec + 1) * 128],
                                        xtl[dc],
                                        start=(dc == 0),
                                        stop=(dc == NDC - 1),
                                    )
                                nc.vector.tensor_copy(dest[ec][:, isl], p)
                        for ii in range(4):
                            lc = 4 * ic + ii
                            p = pp.tile([128, E], F32, tag="pp", name="ppv")
                            for dc in range(NDC):
                                nc.tensor.matmul(
                                    p,
                                    xtl[dc][:, ii * 128 : (ii + 1) * 128],
                                    w_r["v"][dc],
                                    start=(dc == 0),
                                    stop=(dc == NDC - 1),
                                )
                            for hh in range(HPC):
                                nc.vector.tensor_copy(
                                    v[:, lc, voff[hh] : voff[hh] + 64],
                                    p[:, hh * 64 : (hh + 1) * 64],
                                )

                    load_w("k", wk)
                    xtl_cur = load_x(0)
                    load_w("q", wq)
                    load_w("v", wv)
                    load_w("u", wu)
                    for ec in range(2):
                        t = wol.tile([128, D], F32, tag="wol", name="woland")
                        nc.sync.dma_start(out=t, in_=wo[ec * 128 : (ec + 1) * 128, :])
                        nc.vector.tensor_copy(wo_r[ec], t)

                    xtl_next = load_x(1)
                    p1(0, xtl_cur)
                    xtl_cur, xtl_next = xtl_next, load_x(2)
                    p1(1, xtl_cur)
                    attn_tiles(0, 0, range(0, 8))
                    xtl_cur, xtl_next = xtl_next, load_x(3)
                    p1(2, xtl_cur)
                    attn_tiles(0, 0, range(8, 12))
                    p1(3, xtl_next)
                    attn_tiles(0, 0, range(12, 16))

                # ---- rest of attention + fused W_o ----
                with tc.tile_pool(name="ps_c", bufs=1, space="PSUM") as ps_c, \
                     tc.tile_pool(name="wps", bufs=1, space="PSUM") as wps:
                    gate(0, 0, ps_c)
                    attn_tiles(1, 0, range(NJC))
                    gate(1, 0, ps_c)
                    wo_ic(0, wps)
                    for ic in range(1, NIC):
                        dm.clear()
                        for ec in range(2):
                            attn_tiles(ec, ic, range(NJC))
                            gate(ec, ic, ps_c)
                        wo_ic(ic, wps)

    nc.compile()
    return nc


def _host_inputs(x, token_types, seq_lens, W_q, W_k, W_v, W_u, W_o):
    x = np.asarray(x, dtype=np.float32)
    token_types = np.asarray(token_types)
    seq_lens = np.asarray(seq_lens)
    W_q = np.asarray(W_q, dtype=np.float32)
    W_k = np.asarray(W_k, dtype=np.float32)
    W_v = np.asarray(W_v, dtype=np.float32)
    W_u = np.asarray(W_u, dtype=np.float32)
    W_o = np.asarray(W_o, dtype=np.float32)

    per_batch = []
    jr = np.arange(L)
    for b in range(B):
        xt = np.ascontiguousarray(x[b].T)
        prompt = token_types[b] < 3
        valid = jr < int(seq_lens[b])
        ab = np.where(prompt & valid, 0.0, NEG).astype(np.float32)
        bl = np.where(valid, 0.0, NEG).astype(np.float32)
        biasab = np.ascontiguousarray(ab.reshape(NJC, 128).T)
        biasbl = np.ascontiguousarray(bl.reshape(NJC, 128).T)
        dmk = np.empty((NJC, 128, 512), np.float32)
        for jc in range(NJC):
            j = jr[jc * 128 : (jc + 1) * 128]
            i = np.arange((jc // 4) * 512, (jc // 4) * 512 + 512)
            allowed = valid[j][:, None] & (prompt[j][:, None] | (j[:, None] <= i[None, :]))
            dmk[jc] = np.where(allowed, 0.0, NEG)
        per_batch.append((xt, biasab, biasbl, dmk))

    in_maps = []
    for c in range(8):
        b, gi = c // 4, c % 4
        e0 = E * gi
        xt, biasab, biasbl, dmk = per_batch[b]
        in_maps.append(
            {
                "xt": xt,
                "wq": np.ascontiguousarray((W_q[e0 : e0 + E] / 8.0).T),
                "wk": np.ascontiguousarray(W_k[e0 : e0 + E].T),
                "wv": np.ascontiguousarray(W_v[e0 : e0 + E].T),
                "wu": np.ascontiguousarray(W_u[e0 : e0 + E].T),
                "wo": np.ascontiguousarray(W_o[:, e0 : e0 + E].T),
                "biasab": biasab,
                "biasbl": biasbl,
                "dmask": dmk,
            }
        )
    return in_maps


def kernel(x, token_types, seq_lens, W_q, W_k, W_v, W_u, W_o, **_run_kwargs):
    if "nc" not in _cache:
        _cache["nc"] = _build()
    nc = _cache["nc"]
    in_maps = _host_inputs(x, token_types, seq_lens, W_q, W_k, W_v, W_u, W_o)
    try:
        res = run_bass_kernel_spmd(nc, in_maps, list(range(8)), **_run_kwargs)
    except Exception as ex:  # transient NRT device wedge: retry once
        if "UNRECOVERABLE" not in str(ex) and "UNAVAILABLE" not in str(ex):
            raise
        res = run_bass_kernel_spmd(nc, in_maps, list(range(8)), **_run_kwargs)
    _cache["last_result"] = res
    full = np.zeros((B, L, D), np.float64)
    for c in range(8):
        full[c // 4] += res.results[c]["out"].astype(np.float64)
    return full.astype(np.float32)



# revision 7
# speedup vs baseline: 1.3774x; 1.3774x over previous
"""HSTU attention (B=2, L=2048, D=1024, H=16) on 8 TRN2 NeuronCores.

Sharding: heads across cores (2 heads = 128 features per core), both
batches on every core, W_o row-sharded; host sums the 8 partial outputs.

Per batch, keys are sorted on the host into [valid prompt | valid items
by position] (padding dropped).  In sorted order the hybrid mask becomes
a single monotone staircase over per-key thresholds (0 for prompt keys,
original position for item keys), so (key-tile, query-chunk) score tiles
fall into three classes: skipped (fully masked), dense (no mask), or
staircase (bf16 multiplicative 0/1 mask applied to exp(S)).  K/V
projections only cover valid keys.

Everything runs in bf16: projections, scores S^T (both heads of a key
tile paired into one [128,1024] psum / one exp), exp on ACT with bf16
output, AV flipped to O[i, dk] with the e-tile as the stationary operand
(65-row matmuls; a ones column in V yields softmax row sums), one fused
scalar_tensor_tensor per gate tile, PE-transpose of the gated output,
and the row-sharded W_o partial projection.  Staircase mask muls ride on
the Pool engine (SBUF-only); psum evictions split across DVE and ACT.
"""

import sys

for _p in ("/opt/trn_rl_repo", "/root/.axon_site/_ro/trn_rl_repo"):
    if _p not in sys.path:
        sys.path.insert(0, _p)

import numpy as np
import ml_dtypes

import concourse.bass as bass  # noqa: F401
import concourse.mybir as mybir
import concourse.tile as tile
from concourse import bacc
from concourse.bass_utils import run_bass_kernel_spmd

F32 = mybir.dt.float32
BF16 = mybir.dt.bfloat16
EXP = mybir.ActivationFunctionType.Exp
CPY = mybir.ActivationFunctionType.Copy
MUL = mybir.AluOpType.mult
BF = ml_dtypes.bfloat16

B, L, D, H = 2, 2048, 1024, 16
HPC = 2              # heads per core
E = HPC * 64         # 128 features per core
NDC = D // 128       # 8 contraction chunks
NIC = L // 512       # 4 query chunks of 512
BIG = 10 ** 9

_cache = {}


def _mk_schedule(token_types, seq_lens):
    """Host-side key sort + tile classification. Returns (sig, sched, masks)."""
    jr = np.arange(L)
    sched = []
    masks = []
    for b in range(B):
        tt = np.asarray(token_types[b])
        sl = int(seq_lens[b])
        valid = jr < sl
        pidx = jr[(tt < 3) & valid]
        iidx = jr[(tt >= 3) & valid]
        nv = len(pidx) + len(iidx)
        nk = -(-nv // 128)
        npad = nk * 128 - nv
        perm = np.concatenate([pidx, iidx, np.zeros(npad, np.int64)])
        thresh = np.concatenate(
            [np.zeros(len(pidx), np.int64), iidx, np.full(npad, BIG, np.int64)]
        )
        live = [[] for _ in range(NIC)]       # (t, lo_col, mask_idx|None)
        for c in range(NIC):
            i0, i1 = 512 * c, 512 * (c + 1)
            for t in range(nk):
                t_lo = int(thresh[128 * t])
                t_hi = int(thresh[128 * t + 127])
                if t_lo >= i1:
                    continue
                lo = max(0, (t_lo - i0) // 128 * 128)
                if t_hi <= i0:
                    live[c].append((t, 0, None))
                else:
                    m = (
                        (i0 + np.arange(512)[None, :])
                        >= thresh[128 * t : 128 * t + 128, None]
                    ).astype(np.float32)
                    masks.append(m)
                    live[c].append((t, lo, len(masks) - 1))
        avfl = [[None] * 4 for _ in range(NIC)]   # first/last tile per (c, s)
        for c in range(NIC):
            for s in range(4):
                ts = [t for (t, lo, _mi) in live[c] if lo // 128 <= s]
                if ts:
                    avfl[c][s] = (ts[0], ts[-1])
        kchunks = []
        off = 0
        while off < nk * 128:
            sz = min(512, nk * 128 - off)
            kchunks.append((off, sz))
            off += sz
        sched.append(
            dict(nk=nk, nv=nv, perm=perm, live=live, avfl=avfl, kchunks=kchunks)
        )
    nm = max(1, len(masks))
    masks_np = np.zeros((128, nm, 512), BF)
    for i, m in enumerate(masks):
        masks_np[:, i, :] = m.astype(BF)
    sig = tuple(
        (
            s["nk"],
            tuple(
                tuple((t, lo, mi is not None) for (t, lo, mi) in s["live"][c])
                for c in range(NIC)
            ),
        )
        for s in sched
    )
    return sig, sched, masks_np


def _build(sched, nm):
    nc = bacc.Bacc("TRN2", target_bir_lowering=False, debug=False)

    nk = [sched[b]["nk"] for b in range(B)]
    nkc = [len(sched[b]["kchunks"]) for b in range(B)]
    xd = [
        nc.dram_tensor(f"x{b}", [NIC, 128, NDC, 512], BF16, kind="ExternalInput").ap()
        for b in range(B)
    ]
    xkd = [
        nc.dram_tensor(f"xk{b}", [nkc[b], 128, NDC, 512], BF16, kind="ExternalInput").ap()
        for b in range(B)
    ]
    wd = {
        w: nc.dram_tensor(w, [128, NDC, E], BF16, kind="ExternalInput").ap()
        for w in ("wq", "wk", "wv", "wu")
    }
    wod = nc.dram_tensor("wo", [128, D], BF16, kind="ExternalInput").ap()
    idd = nc.dram_tensor("ident", [128, 128], BF16, kind="ExternalInput").ap()
    mkd = nc.dram_tensor("masks", [128, nm, 512], BF16, kind="ExternalInput").ap()
    outd = nc.dram_tensor("outp", [B, NIC, 128, 4, D], BF16, kind="ExternalOutput").ap()

    with tile.TileContext(nc) as tc:
        with tc.tile_pool(name="persist", bufs=1) as pp:
            kt = [pp.tile([128, nk[b] * 128], BF16, tag=f"kt{b}", name=f"kt{b}") for b in range(B)]
            qt = [pp.tile([128, L], BF16, tag=f"qt{b}", name=f"qt{b}") for b in range(B)]
            vt = [
                [pp.tile([128, nk[b], 65], BF16, tag=f"v{b}{h}", name=f"v{b}{h}") for h in range(HPC)]
                for b in range(B)
            ]
            ut = [pp.tile([128, 16, E], BF16, tag=f"u{b}", name=f"u{b}") for b in range(B)]
            wt = {
                w: pp.tile([128, NDC, E], BF16, tag=w, name=f"{w}t")
                for w in ("wq", "wk", "wv", "wu")
            }
            wot = pp.tile([128, D], BF16, tag="wo", name="wot")
            idt = pp.tile([128, 128], BF16, tag="id", name="idt")
            mkt = pp.tile([128, nm, 512], BF16, tag="mk", name="mkt")
            zt = pp.tile([1, 512], BF16, tag="zt", name="zt")
            nc.vector.memset(zt, 0.0)

            for w in ("wk", "wv", "wq", "wu"):
                nc.sync.dma_start(out=wt[w], in_=wd[w])
            nc.sync.dma_start(out=wot, in_=wod)
            nc.sync.dma_start(out=idt, in_=idd)
            nc.sync.dma_start(out=mkt, in_=mkd)
            for b in range(B):
                for h in range(HPC):
                    nc.vector.memset(vt[b][h][:, :, 64:65], 1.0)

            with tc.tile_pool(name="xs", bufs=3) as xs, \
                 tc.tile_pool(name="epool", bufs=4) as ep, \
                 tc.tile_pool(name="gpool", bufs=2) as gp, \
                 tc.tile_pool(name="rpool", bufs=2) as rp, \
                 tc.tile_pool(name="stage", bufs=2) as stp, \
                 tc.tile_pool(name="ps_sp", bufs=2, space="PSUM") as ps_sp, \
                 tc.tile_pool(name="ps_av", bufs=1, space="PSUM") as ps_av, \
                 tc.tile_pool(name="ps_m1", bufs=1, space="PSUM") as ps_m1, \
                 tc.tile_pool(name="ps_m2", bufs=1, space="PSUM") as ps_m2:

                misc_state = [0]

                def misc_pool():
                    misc_state[0] ^= 1
                    return ps_m1 if misc_state[0] else ps_m2

                def load_chunk(dram_ap):
                    t = xs.tile([128, NDC, 512], BF16, tag="xc", name="xc")
                    nc.sync.dma_start(out=t, in_=dram_ap)
                    return t

                # ---------------- projection work items ----------------
                def emit_kv(b, ci):
                    """One xk chunk: K columns + the V tiles inside it."""
                    off, sz = sched[b]["kchunks"][ci]
                    xkc = load_chunk(xkd[b][ci])
                    p = misc_pool().tile([128, 512], F32, tag="m", name="kp")
                    for dc in range(NDC):
                        nc.tensor.matmul(
                            p[:, 0:sz], wt["wk"][:, dc, :], xkc[:, dc, 0:sz],
                            start=(dc == 0), stop=(dc == NDC - 1),
                        )
                    with nc.allow_low_precision(reason="bf16 K"):
                        nc.vector.tensor_copy(kt[b][:, off : off + sz], p[:, 0:sz])
                    for tl in range(sz // 128):
                        tg = off // 128 + tl
                        pv = misc_pool().tile([128, 512], F32, tag="m", name="vp")
                        for dc in range(NDC):
                            nc.tensor.matmul(
                                pv[:, 0:E],
                                xkc[:, dc, 128 * tl : 128 * (tl + 1)],
                                wt["wv"][:, dc, :],
                                start=(dc == 0), stop=(dc == NDC - 1),
                            )
                        with nc.allow_low_precision(reason="bf16 V"):
                            nc.vector.tensor_copy(vt[b][0][:, tg, 0:64], pv[:, 0:64])
                            nc.vector.tensor_copy(vt[b][1][:, tg, 0:64], pv[:, 64:128])

                def emit_qu(b, c):
                    """One x chunk: Q columns + the 4 U tiles inside it."""
                    xc = load_chunk(xd[b][c])
                    p = misc_pool().tile([128, 512], F32, tag="m", name="qp")
                    for dc in range(NDC):
                        nc.tensor.matmul(
                            p, wt["wq"][:, dc, :], xc[:, dc, :],
                            start=(dc == 0), stop=(dc == NDC - 1),
                        )
                    with nc.allow_low_precision(reason="bf16 Q"):
                        nc.vector.tensor_copy(qt[b][:, 512 * c : 512 * (c + 1)], p)
                    for k in range(4):
                        pu = misc_pool().tile([128, 512], F32, tag="m", name="up")
                        for dc in range(NDC):
                            nc.tensor.matmul(
                                pu[:, 0:E],
                                xc[:, dc, 128 * k : 128 * (k + 1)],
                                wt["wu"][:, dc, :],
                                start=(dc == 0), stop=(dc == NDC - 1),
                            )
                        with nc.allow_low_precision(reason="bf16 U"):
                            nc.vector.tensor_copy(ut[b][:, 4 * c + k, :], pu[:, 0:E])

                # ---------------- W_o work items ----------------
                def emit_wo(g, stg, k):
                    gtp = misc_pool().tile([128, 128], BF16, tag="m", name="gtp")
                    nc.tensor.transpose(gtp, g, idt)
                    gts = gp.tile([128, 128], BF16, tag="gts", name="gts")
                    nc.vector.tensor_copy(gts, gtp)
                    for fc in range(2):
                        wp = misc_pool().tile([128, 512], F32, tag="m", name="wp")
                        nc.tensor.matmul(
                            wp, gts, wot[:, 512 * fc : 512 * (fc + 1)],
                            start=True, stop=True,
                        )
                        dst = stg[:, k, 512 * fc : 512 * (fc + 1)]
                        with nc.allow_low_precision(reason="bf16 out"):
                            if fc == 0:
                                nc.vector.tensor_copy(dst, wp)
                            else:
                                nc.scalar.activation(dst, wp, CPY)

                # ---------------- filler queue ----------------
                filler_q = []          # (group, fn) in dependency-safe order
                remaining = {}

                def push(group, fn):
                    filler_q.append((group, fn))
                    remaining[group] = remaining.get(group, 0) + 1

                def pop_one():
                    if filler_q:
                        group, fn = filler_q.pop(0)
                        remaining[group] -= 1
                        fn()

                def drain(group):
                    while remaining.get(group, 0) > 0:
                        pop_one()

                # ---------------- attention ----------------
                def attention(b, c):
                    sb = sched[b]
                    liv = sb["live"][c]
                    avfl = sb["avfl"][c]
                    av = [
                        ps_av.tile([128, 4, 128], F32, tag=f"av{h}", name=f"av{h}")
                        for h in range(HPC)
                    ]
                    t_last = liv[-1][0]
                    for h in range(HPC):
                        # psum start zeroes lazily at whole-bank granularity, so
                        # interleaved sub-chunk groups must share ONE group per
                        # bank: open it with an explicit zeroing outer product.
                        nc.tensor.matmul(
                            av[h][:, :, :],
                            zt[0:1, 0:128], zt[0:1, :],
                            start=True, stop=False, skip_group_check=True,
                        )
                    prev = None

                    def do_av(t, lo, et):
                        for s in range(lo // 128, 4):
                            for h in range(HPC):
                                nc.tensor.matmul(
                                    av[h][:, s, 0:65],
                                    et[:, 512 * h + 128 * s : 512 * h + 128 * (s + 1)],
                                    vt[b][h][:, t, :],
                                    start=False,
                                    stop=(t == t_last and s == 3),
                                    skip_group_check=True,
                                )

                    for t, lo, mi in liv:
                        sp = ps_sp.tile([128, 1024], F32, tag="sp", name="sp")
                        et = ep.tile([128, 1024], BF16, tag="e", name="et")
                        for h in range(HPC):
                            nc.tensor.matmul(
                                sp[:, 512 * h + lo : 512 * h + 512],
                                kt[b][64 * h : 64 * h + 64, 128 * t : 128 * (t + 1)],
                                qt[b][64 * h : 64 * h + 64, 512 * c + lo : 512 * (c + 1)],
                                start=True, stop=True,
                            )
                        if lo == 0:
                            nc.scalar.activation(et, sp, EXP)
                        else:
                            nc.scalar.activation(et[:, lo:512], sp[:, lo:512], EXP)
                            nc.scalar.activation(
                                et[:, 512 + lo : 1024], sp[:, 512 + lo : 1024], EXP
                            )
                        if mi is not None:
                            for h in range(HPC):
                                a0 = 512 * h + lo
                                nc.gpsimd.tensor_tensor(
                                    et[:, a0 : 512 * h + 512],
                                    et[:, a0 : 512 * h + 512],
                                    mkt[:, mi, lo:512],
                                    MUL,
                                )
                        pop_one()
                        if prev is not None:
                            do_av(*prev)
                        prev = (t, lo, et)
                    do_av(*prev)
                    # gating: g = (AV * 1/rowsum) * U, one fused op per (h, s)
                    rec = rp.tile([128, 4, HPC], F32, tag="rec", name="rec")
                    for h in range(HPC):
                        nc.vector.reciprocal(rec[:, :, h : h + 1], av[h][:, :, 64:65])
                    gs = []
                    for s in range(4):
                        g = gp.tile([128, 128], BF16, tag="g", name="g", bufs=8)
                        for h in range(HPC):
                            if avfl[s] is None:
                                nc.vector.memset(g[:, 64 * h : 64 * h + 64], 0.0)
                                continue
                            with nc.allow_low_precision(reason="bf16 gate"):
                                nc.vector.scalar_tensor_tensor(
                                    g[:, 64 * h : 64 * h + 64],
                                    av[h][:, s, 0:64],
                                    rec[:, s, h : h + 1],
                                    ut[b][:, 4 * c + s, 64 * h : 64 * h + 64],
                                    MUL,
                                    MUL,
                                )
                        gs.append(g)
                    return gs

                # ---------------- top-level schedule ----------------
                # phase A: b0 projections needed for (0,0)
                for ci in range(nkc[0]):
                    emit_kv(0, ci)
                emit_qu(0, 0)

                for c in range(1, NIC):
                    push((0, c), lambda c=c: emit_qu(0, c))
                for ci in range(nkc[1]):
                    push((1, 0), lambda ci=ci: emit_kv(1, ci))
                for c in range(NIC):
                    push((1, c), lambda c=c: emit_qu(1, c))

                wo_items = []

                def flush_wo(n):
                    while wo_items and n > 0:
                        wo_items.pop(0)()
                        n -= 1

                for b in range(B):
                    for c in range(NIC):
                        drain((b, c))
                        gs = attention(b, c)
                        stg = stp.tile([128, 4, D], BF16, tag="st", name="stg")
                        for k in range(4):
                            wo_items.append(
                                lambda g=gs[k], stg=stg, k=k: emit_wo(g, stg, k)
                            )
                        wo_items.append(
                            lambda b=b, c=c, stg=stg: nc.sync.dma_start(
                                out=outd[b][c], in_=stg
                            )
                        )
                        if (b, c) != (0, 0):
                            flush_wo(5)
                while filler_q:
                    pop_one()
                flush_wo(len(wo_items))

    nc.compile()
    return nc


def _host_inputs(x, token_types, seq_lens, W_q, W_k, W_v, W_u, W_o, sched, masks_np):
    x = np.asarray(x, dtype=np.float32)
    W = {
        "wq": np.asarray(W_q, np.float32) / 8.0,
        "wk": np.asarray(W_k, np.float32),
        "wv": np.asarray(W_v, np.float32),
        "wu": np.asarray(W_u, np.float32),
    }
    Wo = np.asarray(W_o, np.float32)
    shared = {"ident": np.eye(128, dtype=BF), "masks": masks_np}
    for b in range(B):
        xb = x[b].astype(BF)  # [L, D]
        shared[f"x{b}"] = np.ascontiguousarray(
            xb.reshape(NIC, 512, NDC, 128).transpose(0, 3, 2, 1)
        )  # [c, p, dc, l']
        xkb = xb[sched[b]["perm"]].copy()  # [nk*128, D]
        xkb[sched[b]["nv"] :] = 0
        nch = len(sched[b]["kchunks"])
        xkp = np.zeros((nch * 512, D), BF)
        xkp[: xkb.shape[0]] = xkb
        shared[f"xk{b}"] = np.ascontiguousarray(
            xkp.reshape(nch, 512, NDC, 128).transpose(0, 3, 2, 1)
        )  # [ci, p, dc, j']
    in_maps = []
    for core in range(8):
        e0 = E * core
        im = dict(shared)
        for w, Wm in W.items():
            im[w] = np.ascontiguousarray(
                Wm[e0 : e0 + E].astype(BF).reshape(E, NDC, 128).transpose(2, 1, 0)
            )  # [p, dc, e]
        im["wo"] = np.ascontiguousarray(Wo[:, e0 : e0 + E].astype(BF).T)  # [p, d]
        in_maps.append(im)
    return in_maps


def kernel(x, token_types, seq_lens, W_q, W_k, W_v, W_u, W_o, **_run_kwargs):
    sig, sched, masks_np = _mk_schedule(np.asarray(token_types), np.asarray(seq_lens))
    if _cache.get("sig") != sig:
        _cache["nc"] = _build(sched, masks_np.shape[1])
        _cache["sig"] = sig
    nc = _cache["nc"]
    in_maps = _host_inputs(
        x, token_types, seq_lens, W_q, W_k, W_v, W_u, W_o, sched, masks_np
    )
    try:
        res = run_bass_kernel_spmd(nc, in_maps, list(range(8)), **_run_kwargs)
    except Exception as ex:
        if "UNRECOVERABLE" not in str(ex) and "UNAVAILABLE" not in str(ex):
            raise
        res = run_bass_kernel_spmd(nc, in_maps, list(range(8)), **_run_kwargs)
    _cache["last_result"] = res
    full = np.zeros((B, L, D), np.float64)
    for core in range(8):
        o = res.results[core]["outp"].astype(np.float64)  # [b, c, p, k, d]
        full += o.transpose(0, 1, 3, 2, 4).reshape(B, L, D)
    return full.astype(np.float32)


# revision 13
# speedup vs baseline: 1.4888x; 1.0809x over previous
"""HSTU attention (B=2, L=2048, D=1024, H=16) on 8 TRN2 NeuronCores.

Sharding: heads across cores (2 heads = 128 features per core), both
batches on every core, W_o row-sharded; host sums the 8 partial outputs.

Per batch, keys are sorted on the host into [valid prompt | valid items
by position] (padding dropped).  In sorted order the hybrid mask becomes
a single monotone staircase over per-key thresholds (0 for prompt keys,
original position for item keys), so (key-tile, query-chunk) score tiles
fall into three classes: skipped (fully masked), dense (no mask), or
staircase (bf16 multiplicative 0/1 mask applied to exp(S)).  K/V
projections only cover valid keys.

Everything runs in bf16: projections, scores S^T (both heads of a key
tile paired into one [128,1024] psum / one exp), exp on ACT with bf16
output, AV flipped to O[i, dk] with the e-tile as the stationary operand
(65-row matmuls; a ones column in V yields softmax row sums), one fused
scalar_tensor_tensor per gate tile, PE-transpose of the gated output,
and the row-sharded W_o partial projection.  Staircase mask muls ride on
the Pool engine (SBUF-only); psum evictions split across DVE and ACT.
"""

import sys

for _p in ("/opt/trn_rl_repo", "/root/.axon_site/_ro/trn_rl_repo"):
    if _p not in sys.path:
        sys.path.insert(0, _p)

import numpy as np
import ml_dtypes

import concourse.bass as bass  # noqa: F401
import concourse.mybir as mybir
import concourse.tile as tile
from concourse import bacc
from concourse.bass_utils import run_bass_kernel_spmd

F32 = mybir.dt.float32
BF16 = mybir.dt.bfloat16
EXP = mybir.ActivationFunctionType.Exp
CPY = mybir.ActivationFunctionType.Copy
MUL = mybir.AluOpType.mult
BF = ml_dtypes.bfloat16

B, L, D, H = 2, 2048, 1024, 16
HPC = 2              # heads per core
E = HPC * 64         # 128 features per core
NDC = D // 128       # 8 contraction chunks
NIC = L // 512       # 4 query chunks of 512
BIG = 10 ** 9

_cache = {}


def _mk_schedule(token_types, seq_lens):
    """Host-side key sort + tile classification. Returns (sig, sched, masks)."""
    jr = np.arange(L)
    sched = []
    masks = []
    for b in range(B):
        tt = np.asarray(token_types[b])
        sl = int(seq_lens[b])
        valid = jr < sl
        pidx = jr[(tt < 3) & valid]
        iidx = jr[(tt >= 3) & valid]
        nv = len(pidx) + len(iidx)
        nk = -(-nv // 128)
        npad = nk * 128 - nv
        perm = np.concatenate([pidx, iidx, np.zeros(npad, np.int64)])
        thresh = np.concatenate(
            [np.zeros(len(pidx), np.int64), iidx, np.full(npad, BIG, np.int64)]
        )
        live = [[] for _ in range(NIC)]       # (t, lo_col, mask_idx|None)
        for c in range(NIC):
            i0, i1 = 512 * c, 512 * (c + 1)
            for t in range(nk):
                t_lo = int(thresh[128 * t])
                t_hi = int(thresh[128 * t + 127])
                if t_lo >= i1:
                    continue
                lo = max(0, (t_lo - i0) // 128 * 128)
                if t_hi <= i0:
                    live[c].append((t, 0, None))
                else:
                    m = (
                        (i0 + np.arange(512)[None, :])
                        >= thresh[128 * t : 128 * t + 128, None]
                    ).astype(np.float32)
                    masks.append(m)
                    live[c].append((t, lo, len(masks) - 1))
        avfl = [[None] * 4 for _ in range(NIC)]   # first/last tile per (c, s)
        for c in range(NIC):
            for s in range(4):
                ts = [t for (t, lo, _mi) in live[c] if lo // 128 <= s]
                if ts:
                    avfl[c][s] = (ts[0], ts[-1])
        kchunks = []
        off = 0
        while off < nk * 128:
            sz = min(512, nk * 128 - off)
            kchunks.append((off, sz))
            off += sz
        sched.append(
            dict(nk=nk, nv=nv, perm=perm, live=live, avfl=avfl, kchunks=kchunks)
        )
    nm = max(1, len(masks))
    masks_np = np.zeros((128, nm, 512), BF)
    for i, m in enumerate(masks):
        masks_np[:, i, :] = m.astype(BF)
    sig = tuple(
        (
            s["nk"],
            tuple(
                tuple((t, lo, mi is not None) for (t, lo, mi) in s["live"][c])
                for c in range(NIC)
            ),
        )
        for s in sched
    )
    return sig, sched, masks_np


def _build(sched, nm):
    nc = bacc.Bacc("TRN2", target_bir_lowering=False, debug=False)

    nk = [sched[b]["nk"] for b in range(B)]
    nkc = [len(sched[b]["kchunks"]) for b in range(B)]
    xd = [
        nc.dram_tensor(f"x{b}", [NIC, 128, NDC, 512], BF16, kind="ExternalInput").ap()
        for b in range(B)
    ]
    xkd = [
        nc.dram_tensor(f"xk{b}", [nkc[b], 128, NDC, 512], BF16, kind="ExternalInput").ap()
        for b in range(B)
    ]
    wd = {
        w: nc.dram_tensor(w, [128, NDC, E], BF16, kind="ExternalInput").ap()
        for w in ("wq", "wk", "wv", "wu")
    }
    wod = nc.dram_tensor("wo", [128, D], BF16, kind="ExternalInput").ap()
    idd = nc.dram_tensor("ident", [128, 128], BF16, kind="ExternalInput").ap()
    mkd = nc.dram_tensor("masks", [128, nm, 512], BF16, kind="ExternalInput").ap()
    outd = nc.dram_tensor("outp", [B, NIC, 128, 4, D], BF16, kind="ExternalOutput").ap()

    with tile.TileContext(nc) as tc:
        with tc.tile_pool(name="persist", bufs=1) as pp:
            kt = [pp.tile([128, nk[b] * 128], BF16, tag=f"kt{b}", name=f"kt{b}") for b in range(B)]
            qt = [pp.tile([128, L], BF16, tag=f"qt{b}", name=f"qt{b}") for b in range(B)]
            vt = [
                [pp.tile([128, nk[b], 65], BF16, tag=f"v{b}{h}", name=f"v{b}{h}") for h in range(HPC)]
                for b in range(B)
            ]
            ut = [pp.tile([128, 16, E], BF16, tag=f"u{b}", name=f"u{b}") for b in range(B)]
            wt = {
                w: pp.tile([128, NDC, E], BF16, tag=w, name=f"{w}t")
                for w in ("wq", "wk", "wv", "wu")
            }
            wot = pp.tile([128, D], BF16, tag="wo", name="wot")
            idt = pp.tile([128, 128], BF16, tag="id", name="idt")
            mkt = pp.tile([128, nm, 512], BF16, tag="mk", name="mkt")
            zt = pp.tile([1, 512], BF16, tag="zt", name="zt")
            nc.vector.memset(zt, 0.0)

            for w in ("wk", "wv", "wq", "wu"):
                nc.sync.dma_start(out=wt[w], in_=wd[w])
            nc.sync.dma_start(out=wot, in_=wod)
            nc.sync.dma_start(out=idt, in_=idd)
            for b in range(B):
                for h in range(HPC):
                    nc.vector.memset(vt[b][h][:, :, 64:65], 1.0)

            with tc.tile_pool(name="xs", bufs=3) as xs, \
                 tc.tile_pool(name="epool", bufs=4) as ep, \
                 tc.tile_pool(name="gpool", bufs=2) as gp, \
                 tc.tile_pool(name="rpool", bufs=2) as rp, \
                 tc.tile_pool(name="stage", bufs=2) as stp, \
                 tc.tile_pool(name="ps_sp", bufs=2, space="PSUM") as ps_sp, \
                 tc.tile_pool(name="ps_av", bufs=1, space="PSUM") as ps_av, \
                 tc.tile_pool(name="ps_m1", bufs=1, space="PSUM") as ps_m1, \
                 tc.tile_pool(name="ps_m2", bufs=1, space="PSUM") as ps_m2:

                misc_state = [0]

                def misc_pool():
                    misc_state[0] ^= 1
                    return ps_m1 if misc_state[0] else ps_m2

                chunk_tiles = {}

                def load_chunk(key, dram_ap):
                    t = xs.tile([128, NDC, 512], BF16, tag="xc", name="xc")
                    nc.sync.dma_start(out=t, in_=dram_ap)
                    chunk_tiles[key] = t

                # ---------------- projection work items ----------------
                def emit_k(b, ci):
                    off, sz = sched[b]["kchunks"][ci]
                    xkc = chunk_tiles[("xk", b, ci)]
                    p = misc_pool().tile([128, 512], F32, tag="m", name="kp")
                    for dc in range(NDC):
                        nc.tensor.matmul(
                            p[:, 0:sz], wt["wk"][:, dc, :], xkc[:, dc, 0:sz],
                            start=(dc == 0), stop=(dc == NDC - 1),
                        )
                    with nc.allow_low_precision(reason="bf16 K"):
                        nc.vector.tensor_copy(kt[b][:, off : off + sz], p[:, 0:sz])

                def emit_v(b, ci, tl):
                    off, _sz = sched[b]["kchunks"][ci]
                    xkc = chunk_tiles[("xk", b, ci)]
                    tg = off // 128 + tl
                    pv = misc_pool().tile([128, 512], F32, tag="m", name="vp")
                    for dc in range(NDC):
                        nc.tensor.matmul(
                            pv[:, 0:E],
                            xkc[:, dc, 128 * tl : 128 * (tl + 1)],
                            wt["wv"][:, dc, :],
                            start=(dc == 0), stop=(dc == NDC - 1),
                        )
                    with nc.allow_low_precision(reason="bf16 V"):
                        nc.vector.tensor_copy(vt[b][0][:, tg, 0:64], pv[:, 0:64])
                        nc.vector.tensor_copy(vt[b][1][:, tg, 0:64], pv[:, 64:128])

                def emit_q(b, c):
                    xc = chunk_tiles[("x", b, c)]
                    p = misc_pool().tile([128, 512], F32, tag="m", name="qp")
                    for dc in range(NDC):
                        nc.tensor.matmul(
                            p, wt["wq"][:, dc, :], xc[:, dc, :],
                            start=(dc == 0), stop=(dc == NDC - 1),
                        )
                    with nc.allow_low_precision(reason="bf16 Q"):
                        nc.vector.tensor_copy(qt[b][:, 512 * c : 512 * (c + 1)], p)

                def emit_u(b, c, k):
                    xc = chunk_tiles[("x", b, c)]
                    pu = misc_pool().tile([128, 512], F32, tag="m", name="up")
                    for dc in range(NDC):
                        nc.tensor.matmul(
                            pu[:, 0:E],
                            xc[:, dc, 128 * k : 128 * (k + 1)],
                            wt["wu"][:, dc, :],
                            start=(dc == 0), stop=(dc == NDC - 1),
                        )
                    with nc.allow_low_precision(reason="bf16 U"):
                        nc.vector.tensor_copy(ut[b][:, 4 * c + k, :], pu[:, 0:E])

                # ---------------- W_o work items ----------------
                def emit_wo(b, c, g, stg, k):
                    gtp = misc_pool().tile([128, 128], BF16, tag="m", name="gtp")
                    nc.tensor.transpose(gtp, g, idt)
                    gts = gp.tile([128, 128], BF16, tag="gts", name="gts")
                    nc.vector.tensor_copy(gts, gtp)
                    for fc in range(2):
                        wp = misc_pool().tile([128, 512], F32, tag="m", name="wp")
                        nc.tensor.matmul(
                            wp, gts, wot[:, 512 * fc : 512 * (fc + 1)],
                            start=True, stop=True,
                        )
                        with nc.allow_low_precision(reason="bf16 out"):
                            nc.vector.tensor_copy(
                                stg[:, k, 512 * fc : 512 * (fc + 1)], wp
                            )
                    nc.sync.dma_start(out=outd[b][c][:, k, :], in_=stg[:, k, :])

                # ---------------- filler queue ----------------
                filler_q = []          # (group, fn) in dependency-safe order
                remaining = {}

                def push(group, fn):
                    filler_q.append((group, fn))
                    remaining[group] = remaining.get(group, 0) + 1

                def push_front(group, fn):
                    filler_q.insert(0, (group, fn))
                    remaining[group] = remaining.get(group, 0) + 1

                def pop_one():
                    if filler_q:
                        group, fn = filler_q.pop(0)
                        remaining[group] -= 1
                        fn()

                def drain(group):
                    while remaining.get(group, 0) > 0:
                        pop_one()

                # ---------------- attention ----------------
                def attention(b, c):
                    sb = sched[b]
                    liv = sb["live"][c]
                    avfl = sb["avfl"][c]
                    av = [
                        ps_av.tile([128, 4, 128], F32, tag=f"av{h}", name=f"av{h}")
                        for h in range(HPC)
                    ]
                    t_last = liv[-1][0]
                    for h in range(HPC):
                        # psum start zeroes lazily at whole-bank granularity, so
                        # interleaved sub-chunk groups must share ONE group per
                        # bank: open it with an explicit zeroing outer product.
                        nc.tensor.matmul(
                            av[h][:, :, :],
                            zt[0:1, 0:128], zt[0:1, :],
                            start=True, stop=False, skip_group_check=True,
                        )
                    prev = None

                    def do_av(t, lo, et):
                        for s in range(lo // 128, 4):
                            for h in range(HPC):
                                nc.tensor.matmul(
                                    av[h][:, s, 0:65],
                                    et[:, 512 * h + 128 * s : 512 * h + 128 * (s + 1)],
                                    vt[b][h][:, t, :],
                                    start=False,
                                    stop=(t == t_last and s == 3),
                                    skip_group_check=True,
                                )

                    for t, lo, mi in liv:
                        sp = ps_sp.tile([128, 1024], F32, tag="sp", name="sp")
                        et = ep.tile([128, 1024], BF16, tag="e", name="et")
                        for h in range(HPC):
                            nc.tensor.matmul(
                                sp[:, 512 * h + lo : 512 * h + 512],
                                kt[b][64 * h : 64 * h + 64, 128 * t : 128 * (t + 1)],
                                qt[b][64 * h : 64 * h + 64, 512 * c + lo : 512 * (c + 1)],
                                start=True, stop=True,
                            )
                        if lo == 0:
                            nc.scalar.activation(et, sp, EXP)
                        else:
                            nc.scalar.activation(et[:, lo:512], sp[:, lo:512], EXP)
                            nc.scalar.activation(
                                et[:, 512 + lo : 1024], sp[:, 512 + lo : 1024], EXP
                            )
                        if mi is not None:
                            for h in range(HPC):
                                a0 = 512 * h + lo
                                nc.gpsimd.tensor_tensor(
                                    et[:, a0 : 512 * h + 512],
                                    et[:, a0 : 512 * h + 512],
                                    mkt[:, mi, lo:512],
                                    MUL,
                                )
                        pop_one()
                        if prev is not None:
                            do_av(*prev)
                        prev = (t, lo, et)
                    do_av(*prev)
                    # gating: g = (AV * 1/rowsum) * U, one fused op per (h, s)
                    rec = rp.tile([128, 4, HPC], F32, tag="rec", name="rec")
                    for h in range(HPC):
                        nc.vector.reciprocal(rec[:, :, h : h + 1], av[h][:, :, 64:65])
                    gs = []
                    for s in range(4):
                        g = gp.tile([128, 128], BF16, tag="g", name="g", bufs=8)
                        for h in range(HPC):
                            if avfl[s] is None:
                                nc.vector.memset(g[:, 64 * h : 64 * h + 64], 0.0)
                                continue
                            with nc.allow_low_precision(reason="bf16 gate"):
                                nc.vector.scalar_tensor_tensor(
                                    g[:, 64 * h : 64 * h + 64],
                                    av[h][:, s, 0:64],
                                    rec[:, s, h : h + 1],
                                    ut[b][:, 4 * c + s, 64 * h : 64 * h + 64],
                                    MUL,
                                    MUL,
                                )
                        gs.append(g)
                    return gs

                # ---------------- top-level schedule ----------------
                # phase A: b0 projections needed for (0,0), emitted directly
                # (load->K->V per chunk so xs-pool slot reuse never waits on
                # consumers that would be emitted later)
                for ci in range(nkc[0]):
                    load_chunk(("xk", 0, ci), xkd[0][ci])
                    emit_k(0, ci)
                    _off, _sz = sched[0]["kchunks"][ci]
                    for tl in range(_sz // 128):
                        emit_v(0, ci, tl)
                load_chunk(("x", 0, 0), xd[0][0])
                emit_q(0, 0)
                for k in range(4):
                    emit_u(0, 0, k)
                nc.sync.dma_start(out=mkt, in_=mkd)

                # filler: remaining b0 Q/U, all b1 projections, fine-grained
                for c in range(1, NIC):
                    push((0, c), lambda c=c: load_chunk(("x", 0, c), xd[0][c]))
                    push((0, c), lambda c=c: emit_q(0, c))
                    for k in range(4):
                        push((0, c), lambda c=c, k=k: emit_u(0, c, k))
                for ci in range(nkc[1]):
                    push((1, 0), lambda ci=ci: load_chunk(("xk", 1, ci), xkd[1][ci]))
                    push((1, 0), lambda ci=ci: emit_k(1, ci))
                    _off, _sz = sched[1]["kchunks"][ci]
                    for tl in range(_sz // 128):
                        push((1, 0), lambda ci=ci, tl=tl: emit_v(1, ci, tl))
                for c in range(NIC):
                    push((1, c), lambda c=c: load_chunk(("x", 1, c), xd[1][c]))
                    push((1, c), lambda c=c: emit_q(1, c))
                    for k in range(4):
                        push((1, c), lambda c=c, k=k: emit_u(1, c, k))

                for b in range(B):
                    for c in range(NIC):
                        drain((b, c))
                        gs = attention(b, c)
                        stg = stp.tile([128, 4, D], BF16, tag="st", name="stg")
                        for k in range(3, -1, -1):
                            push_front(
                                ("wo", b, c),
                                lambda b=b, c=c, g=gs[k], stg=stg, k=k: emit_wo(
                                    b, c, g, stg, k
                                ),
                            )
                while filler_q:
                    pop_one()

    nc.compile()
    return nc


def _host_inputs(x, token_types, seq_lens, W_q, W_k, W_v, W_u, W_o, sched, masks_np):
    x = np.asarray(x, dtype=np.float32)
    W = {
        "wq": np.asarray(W_q, np.float32) / 8.0,
        "wk": np.asarray(W_k, np.float32),
        "wv": np.asarray(W_v, np.float32),
        "wu": np.asarray(W_u, np.float32),
    }
    Wo = np.asarray(W_o, np.float32)
    shared = {"ident": np.eye(128, dtype=BF), "masks": masks_np}
    for b in range(B):
        xb = x[b].astype(BF)  # [L, D]
        shared[f"x{b}"] = np.ascontiguousarray(
            xb.reshape(NIC, 512, NDC, 128).transpose(0, 3, 2, 1)
        )  # [c, p, dc, l']
        xkb = xb[sched[b]["perm"]].copy()  # [nk*128, D]
        xkb[sched[b]["nv"] :] = 0
        nch = len(sched[b]["kchunks"])
        xkp = np.zeros((nch * 512, D), BF)
        xkp[: xkb.shape[0]] = xkb
        shared[f"xk{b}"] = np.ascontiguousarray(
            xkp.reshape(nch, 512, NDC, 128).transpose(0, 3, 2, 1)
        )  # [ci, p, dc, j']
    in_maps = []
    for core in range(8):
        e0 = E * core
        im = dict(shared)
        for w, Wm in W.items():
            im[w] = np.ascontiguousarray(
                Wm[e0 : e0 + E].astype(BF).reshape(E, NDC, 128).transpose(2, 1, 0)
            )  # [p, dc, e]
        im["wo"] = np.ascontiguousarray(Wo[:, e0 : e0 + E].astype(BF).T)  # [p, d]
        in_maps.append(im)
    return in_maps


def kernel(x, token_types, seq_lens, W_q, W_k, W_v, W_u, W_o, **_run_kwargs):
    sig, sched, masks_np = _mk_schedule(np.asarray(token_types), np.asarray(seq_lens))
    if _cache.get("sig") != sig:
        _cache["nc"] = _build(sched, masks_np.shape[1])
        _cache["sig"] = sig
    nc = _cache["nc"]
    in_maps = _host_inputs(
        x, token_types, seq_lens, W_q, W_k, W_v, W_u, W_o, sched, masks_np
    )
    try:
        res = run_bass_kernel_spmd(nc, in_maps, list(range(8)), **_run_kwargs)
    except Exception as ex:
        if "UNRECOVERABLE" not in str(ex) and "UNAVAILABLE" not in str(ex):
            raise
        res = run_bass_kernel_spmd(nc, in_maps, list(range(8)), **_run_kwargs)
    _cache["last_result"] = res
    full = np.zeros((B, L, D), np.float64)
    for core in range(8):
        o = res.results[core]["outp"].astype(np.float64)  # [b, c, p, k, d]
        full += o.transpose(0, 1, 3, 2, 4).reshape(B, L, D)
    return full.astype(np.float32)


# revision 14
# speedup vs baseline: 1.5466x; 1.0388x over previous
"""HSTU attention (B=2, L=2048, D=1024, H=16) on 8 TRN2 NeuronCores.

Sharding: heads across cores (2 heads = 128 features per core), both
batches on every core, W_o row-sharded; host sums the 8 partial outputs.

Per batch, keys are sorted on the host into [valid prompt | valid items
by position] (padding dropped).  In sorted order the hybrid mask becomes
a single monotone staircase over per-key thresholds (0 for prompt keys,
original position for item keys), so (key-tile, query-chunk) score tiles
fall into three classes: skipped (fully masked), dense (no mask), or
staircase (bf16 multiplicative 0/1 mask applied to exp(S)).  K/V
projections only cover valid keys.

Everything runs in bf16: projections, scores S^T (both heads of a key
tile paired into one [128,1024] psum / one exp), exp on ACT with bf16
output, AV flipped to O[i, dk] with the e-tile as the stationary operand
(65-row matmuls; a ones column in V yields softmax row sums), one fused
scalar_tensor_tensor per gate tile, PE-transpose of the gated output,
and the row-sharded W_o partial projection.  Staircase mask muls ride on
the Pool engine (SBUF-only); psum evictions split across DVE and ACT.
"""

import sys

for _p in ("/opt/trn_rl_repo", "/root/.axon_site/_ro/trn_rl_repo"):
    if _p not in sys.path:
        sys.path.insert(0, _p)

import numpy as np
import ml_dtypes

import concourse.bass as bass  # noqa: F401
import concourse.mybir as mybir
import concourse.tile as tile
from concourse import bacc
from concourse.bass_utils import run_bass_kernel_spmd

F32 = mybir.dt.float32
BF16 = mybir.dt.bfloat16
EXP = mybir.ActivationFunctionType.Exp
CPY = mybir.ActivationFunctionType.Copy
MUL = mybir.AluOpType.mult
BF = ml_dtypes.bfloat16

B, L, D, H = 2, 2048, 1024, 16
HPC = 2              # heads per core
E = HPC * 64         # 128 features per core
NDC = D // 128       # 8 contraction chunks
NIC = L // 512       # 4 query chunks of 512
BIG = 10 ** 9

_cache = {}


def _mk_schedule(token_types, seq_lens):
    """Host-side key sort + tile classification. Returns (sig, sched, masks)."""
    jr = np.arange(L)
    sched = []
    masks = []
    for b in range(B):
        tt = np.asarray(token_types[b])
        sl = int(seq_lens[b])
        valid = jr < sl
        pidx = jr[(tt < 3) & valid]
        iidx = jr[(tt >= 3) & valid]
        nv = len(pidx) + len(iidx)
        nk = -(-nv // 128)
        npad = nk * 128 - nv
        perm = np.concatenate([pidx, iidx, np.zeros(npad, np.int64)])
        thresh = np.concatenate(
            [np.zeros(len(pidx), np.int64), iidx, np.full(npad, BIG, np.int64)]
        )
        live = [[] for _ in range(NIC)]       # (t, lo_col, mask_idx|None)
        for c in range(NIC):
            i0, i1 = 512 * c, 512 * (c + 1)
            for t in range(nk):
                t_lo = int(thresh[128 * t])
                t_hi = int(thresh[128 * t + 127])
                if t_lo >= i1:
                    continue
                lo = max(0, (t_lo - i0) // 128 * 128)
                if t_hi <= i0:
                    live[c].append((t, 0, None))
                else:
                    m = (
                        (i0 + np.arange(512)[None, :])
                        >= thresh[128 * t : 128 * t + 128, None]
                    ).astype(np.float32)
                    masks.append(m)
                    live[c].append((t, lo, len(masks) - 1))
        avfl = [[None] * 4 for _ in range(NIC)]   # first/last tile per (c, s)
        for c in range(NIC):
            for s in range(4):
                ts = [t for (t, lo, _mi) in live[c] if lo // 128 <= s]
                if ts:
                    avfl[c][s] = (ts[0], ts[-1])
        kchunks = []
        off = 0
        while off < nk * 128:
            sz = min(512, nk * 128 - off)
            kchunks.append((off, sz))
            off += sz
        sched.append(
            dict(nk=nk, nv=nv, perm=perm, live=live, avfl=avfl, kchunks=kchunks)
        )
    nm = max(1, len(masks))
    masks_np = np.zeros((128, nm, 512), BF)
    for i, m in enumerate(masks):
        masks_np[:, i, :] = m.astype(BF)
    sig = tuple(
        (
            s["nk"],
            tuple(
                tuple((t, lo, mi is not None) for (t, lo, mi) in s["live"][c])
                for c in range(NIC)
            ),
        )
        for s in sched
    )
    return sig, sched, masks_np


def _build(sched, nm):
    nc = bacc.Bacc("TRN2", target_bir_lowering=False, debug=False)

    nk = [sched[b]["nk"] for b in range(B)]
    nkc = [len(sched[b]["kchunks"]) for b in range(B)]
    xd = [
        nc.dram_tensor(f"x{b}", [NIC, 128, NDC, 512], BF16, kind="ExternalInput").ap()
        for b in range(B)
    ]
    xkd = [
        nc.dram_tensor(f"xk{b}", [nkc[b], 128, NDC, 512], BF16, kind="ExternalInput").ap()
        for b in range(B)
    ]
    wd = {
        w: nc.dram_tensor(w, [128, NDC, E], BF16, kind="ExternalInput").ap()
        for w in ("wq", "wk", "wv", "wu")
    }
    wod = nc.dram_tensor("wo", [128, D], BF16, kind="ExternalInput").ap()
    idd = nc.dram_tensor("ident", [128, 128], BF16, kind="ExternalInput").ap()
    mkd = nc.dram_tensor("masks", [128, nm, 512], BF16, kind="ExternalInput").ap()
    outd = nc.dram_tensor("outp", [B, NIC, 128, 4, D], BF16, kind="ExternalOutput").ap()

    with tile.TileContext(nc) as tc:
        with tc.tile_pool(name="persist", bufs=1) as pp:
            kt = [pp.tile([128, nk[b] * 128], BF16, tag=f"kt{b}", name=f"kt{b}") for b in range(B)]
            qt = [pp.tile([128, L], BF16, tag=f"qt{b}", name=f"qt{b}") for b in range(B)]
            vt = [
                [pp.tile([128, nk[b], 65], BF16, tag=f"v{b}{h}", name=f"v{b}{h}") for h in range(HPC)]
                for b in range(B)
            ]
            ut = [pp.tile([128, 16, E], BF16, tag=f"u{b}", name=f"u{b}") for b in range(B)]
            wt = {
                w: pp.tile([128, NDC, E], BF16, tag=w, name=f"{w}t")
                for w in ("wq", "wk", "wv", "wu")
            }
            wot = pp.tile([128, D], BF16, tag="wo", name="wot")
            idt = pp.tile([128, 128], BF16, tag="id", name="idt")
            mkt = pp.tile([128, nm, 512], BF16, tag="mk", name="mkt")
            zt = pp.tile([1, 512], BF16, tag="zt", name="zt")
            nc.vector.memset(zt, 0.0)

            nc.sync.dma_start(out=wt["wk"], in_=wd["wk"])
            for b in range(B):
                for h in range(HPC):
                    nc.vector.memset(vt[b][h][:, :, 64:65], 1.0)

            with tc.tile_pool(name="xs", bufs=6) as xs, \
                 tc.tile_pool(name="epool", bufs=6) as ep, \
                 tc.tile_pool(name="gpool", bufs=2) as gp, \
                 tc.tile_pool(name="rpool", bufs=2) as rp, \
                 tc.tile_pool(name="stage", bufs=2) as stp, \
                 tc.tile_pool(name="ps_sp", bufs=2, space="PSUM") as ps_sp, \
                 tc.tile_pool(name="ps_av", bufs=1, space="PSUM") as ps_av, \
                 tc.tile_pool(name="ps_m1", bufs=1, space="PSUM") as ps_m1, \
                 tc.tile_pool(name="ps_m2", bufs=1, space="PSUM") as ps_m2:

                misc_state = [0]

                def misc_pool():
                    misc_state[0] ^= 1
                    return ps_m1 if misc_state[0] else ps_m2

                chunk_tiles = {}

                def load_chunk(key, dram_ap):
                    t = xs.tile([128, NDC, 512], BF16, tag="xc", name="xc")
                    nc.sync.dma_start(out=t, in_=dram_ap)
                    chunk_tiles[key] = t

                # ---------------- projection work items ----------------
                def emit_k(b, ci):
                    off, sz = sched[b]["kchunks"][ci]
                    xkc = chunk_tiles[("xk", b, ci)]
                    p = misc_pool().tile([128, 512], F32, tag="m", name="kp")
                    for dc in range(NDC):
                        nc.tensor.matmul(
                            p[:, 0:sz], wt["wk"][:, dc, :], xkc[:, dc, 0:sz],
                            start=(dc == 0), stop=(dc == NDC - 1),
                        )
                    with nc.allow_low_precision(reason="bf16 K"):
                        nc.vector.tensor_copy(kt[b][:, off : off + sz], p[:, 0:sz])

                def emit_v(b, ci, tl):
                    off, _sz = sched[b]["kchunks"][ci]
                    xkc = chunk_tiles[("xk", b, ci)]
                    tg = off // 128 + tl
                    pv = misc_pool().tile([128, 512], F32, tag="m", name="vp")
                    for dc in range(NDC):
                        nc.tensor.matmul(
                            pv[:, 0:E],
                            xkc[:, dc, 128 * tl : 128 * (tl + 1)],
                            wt["wv"][:, dc, :],
                            start=(dc == 0), stop=(dc == NDC - 1),
                        )
                    with nc.allow_low_precision(reason="bf16 V"):
                        nc.vector.tensor_copy(vt[b][0][:, tg, 0:64], pv[:, 0:64])
                        nc.vector.tensor_copy(vt[b][1][:, tg, 0:64], pv[:, 64:128])

                def emit_q(b, c):
                    xc = chunk_tiles[("x", b, c)]
                    p = misc_pool().tile([128, 512], F32, tag="m", name="qp")
                    for dc in range(NDC):
                        nc.tensor.matmul(
                            p, wt["wq"][:, dc, :], xc[:, dc, :],
                            start=(dc == 0), stop=(dc == NDC - 1),
                        )
                    with nc.allow_low_precision(reason="bf16 Q"):
                        nc.vector.tensor_copy(qt[b][:, 512 * c : 512 * (c + 1)], p)

                def emit_u(b, c, k):
                    xc = chunk_tiles[("x", b, c)]
                    pu = misc_pool().tile([128, 512], F32, tag="m", name="up")
                    for dc in range(NDC):
                        nc.tensor.matmul(
                            pu[:, 0:E],
                            xc[:, dc, 128 * k : 128 * (k + 1)],
                            wt["wu"][:, dc, :],
                            start=(dc == 0), stop=(dc == NDC - 1),
                        )
                    with nc.allow_low_precision(reason="bf16 U"):
                        nc.vector.tensor_copy(ut[b][:, 4 * c + k, :], pu[:, 0:E])

                # ---------------- W_o work items ----------------
                def emit_wo(b, c, g, stg, k):
                    gtp = misc_pool().tile([128, 128], BF16, tag="m", name="gtp")
                    nc.tensor.transpose(gtp, g, idt)
                    gts = gp.tile([128, 128], BF16, tag="gts", name="gts")
                    nc.vector.tensor_copy(gts, gtp)
                    for fc in range(2):
                        wp = misc_pool().tile([128, 512], F32, tag="m", name="wp")
                        nc.tensor.matmul(
                            wp, gts, wot[:, 512 * fc : 512 * (fc + 1)],
                            start=True, stop=True,
                        )
                        with nc.allow_low_precision(reason="bf16 out"):
                            nc.vector.tensor_copy(
                                stg[:, k, 512 * fc : 512 * (fc + 1)], wp
                            )
                    nc.sync.dma_start(out=outd[b][c][:, k, :], in_=stg[:, k, :])

                # ---------------- filler queue ----------------
                filler_q = []          # (group, fn) in dependency-safe order
                remaining = {}

                def push(group, fn):
                    filler_q.append((group, fn))
                    remaining[group] = remaining.get(group, 0) + 1

                def push_front(group, fn):
                    filler_q.insert(0, (group, fn))
                    remaining[group] = remaining.get(group, 0) + 1

                def pop_one():
                    if filler_q:
                        group, fn = filler_q.pop(0)
                        remaining[group] -= 1
                        fn()

                def drain(group):
                    while remaining.get(group, 0) > 0:
                        pop_one()

                # ---------------- attention ----------------
                tiles_left = [sum(len(sched[bb]["live"][cc]) for bb in range(B) for cc in range(NIC))]

                def attention(b, c):
                    sb = sched[b]
                    liv = sb["live"][c]
                    avfl = sb["avfl"][c]
                    av = [
                        ps_av.tile([128, 4, 128], F32, tag=f"av{h}", name=f"av{h}")
                        for h in range(HPC)
                    ]
                    t_last = liv[-1][0]
                    for h in range(HPC):
                        # psum start zeroes lazily at whole-bank granularity, so
                        # interleaved sub-chunk groups must share ONE group per
                        # bank: open it with an explicit zeroing outer product.
                        nc.tensor.matmul(
                            av[h][:, :, :],
                            zt[0:1, 0:128], zt[0:1, :],
                            start=True, stop=False, skip_group_check=True,
                        )
                    prev = None

                    def do_av(t, lo, et):
                        for s in range(lo // 128, 4):
                            for h in range(HPC):
                                nc.tensor.matmul(
                                    av[h][:, s, 0:65],
                                    et[:, 512 * h + 128 * s : 512 * h + 128 * (s + 1)],
                                    vt[b][h][:, t, :],
                                    start=False,
                                    stop=(t == t_last and s == 3),
                                    skip_group_check=True,
                                )

                    for t, lo, mi in liv:
                        sp = ps_sp.tile([128, 1024], F32, tag="sp", name="sp")
                        et = ep.tile([128, 1024], BF16, tag="e", name="et")
                        for h in range(HPC):
                            nc.tensor.matmul(
                                sp[:, 512 * h + lo : 512 * h + 512],
                                kt[b][64 * h : 64 * h + 64, 128 * t : 128 * (t + 1)],
                                qt[b][64 * h : 64 * h + 64, 512 * c + lo : 512 * (c + 1)],
                                start=True, stop=True,
                            )
                        if lo == 0:
                            nc.scalar.activation(et, sp, EXP)
                        else:
                            nc.scalar.activation(et[:, lo:512], sp[:, lo:512], EXP)
                            nc.scalar.activation(
                                et[:, 512 + lo : 1024], sp[:, 512 + lo : 1024], EXP
                            )
                        if mi is not None:
                            for h in range(HPC):
                                a0 = 512 * h + lo
                                nc.gpsimd.tensor_tensor(
                                    et[:, a0 : 512 * h + 512],
                                    et[:, a0 : 512 * h + 512],
                                    mkt[:, mi, lo:512],
                                    MUL,
                                )
                        pop_one()
                        if len(filler_q) > tiles_left[0]:
                            pop_one()
                        tiles_left[0] -= 1
                        if prev is not None:
                            do_av(*prev)
                        prev = (t, lo, et)
                    do_av(*prev)
                    # gating: g = (AV * 1/rowsum) * U, one fused op per (h, s)
                    rec = rp.tile([128, 4, HPC], F32, tag="rec", name="rec")
                    for h in range(HPC):
                        nc.vector.reciprocal(rec[:, :, h : h + 1], av[h][:, :, 64:65])
                    gs = []
                    for s in range(4):
                        g = gp.tile([128, 128], BF16, tag="g", name="g", bufs=8)
                        for h in range(HPC):
                            if avfl[s] is None:
                                nc.vector.memset(g[:, 64 * h : 64 * h + 64], 0.0)
                                continue
                            with nc.allow_low_precision(reason="bf16 gate"):
                                nc.vector.scalar_tensor_tensor(
                                    g[:, 64 * h : 64 * h + 64],
                                    av[h][:, s, 0:64],
                                    rec[:, s, h : h + 1],
                                    ut[b][:, 4 * c + s, 64 * h : 64 * h + 64],
                                    MUL,
                                    MUL,
                                )
                        gs.append(g)
                    return gs

                # ---------------- top-level schedule ----------------
                # phase A: b0 projections needed for (0,0), emitted directly
                # (load->K->V per chunk so xs-pool slot reuse never waits on
                # consumers that would be emitted later)
                for ci in range(nkc[0]):
                    load_chunk(("xk", 0, ci), xkd[0][ci])
                    if ci == 0:
                        nc.sync.dma_start(out=wt["wv"], in_=wd["wv"])
                        nc.sync.dma_start(out=wt["wq"], in_=wd["wq"])
                        nc.sync.dma_start(out=wt["wu"], in_=wd["wu"])
                    emit_k(0, ci)
                    _off, _sz = sched[0]["kchunks"][ci]
                    for tl in range(_sz // 128):
                        emit_v(0, ci, tl)
                load_chunk(("x", 0, 0), xd[0][0])
                nc.sync.dma_start(out=wot, in_=wod)
                nc.sync.dma_start(out=idt, in_=idd)
                emit_q(0, 0)
                for k in range(4):
                    emit_u(0, 0, k)
                nc.sync.dma_start(out=mkt, in_=mkd)

                # filler: remaining b0 Q/U, all b1 projections, fine-grained
                for c in range(1, NIC):
                    push((0, c), lambda c=c: load_chunk(("x", 0, c), xd[0][c]))
                    push((0, c), lambda c=c: emit_q(0, c))
                    for k in range(4):
                        push((0, c), lambda c=c, k=k: emit_u(0, c, k))
                for ci in range(nkc[1]):
                    push((1, 0), lambda ci=ci: load_chunk(("xk", 1, ci), xkd[1][ci]))
                    push((1, 0), lambda ci=ci: emit_k(1, ci))
                    _off, _sz = sched[1]["kchunks"][ci]
                    for tl in range(_sz // 128):
                        push((1, 0), lambda ci=ci, tl=tl: emit_v(1, ci, tl))
                for c in range(NIC):
                    push((1, c), lambda c=c: load_chunk(("x", 1, c), xd[1][c]))
                    push((1, c), lambda c=c: emit_q(1, c))
                    for k in range(4):
                        push((1, c), lambda c=c, k=k: emit_u(1, c, k))

                for b in range(B):
                    for c in range(NIC):
                        drain((b, c))
                        gs = attention(b, c)
                        stg = stp.tile([128, 4, D], BF16, tag="st", name="stg")
                        for k in range(3, -1, -1):
                            push_front(
                                ("wo", b, c),
                                lambda b=b, c=c, g=gs[k], stg=stg, k=k: emit_wo(
                                    b, c, g, stg, k
                                ),
                            )
                while filler_q:
                    pop_one()

    nc.compile()
    return nc


def _host_inputs(x, token_types, seq_lens, W_q, W_k, W_v, W_u, W_o, sched, masks_np):
    x = np.asarray(x, dtype=np.float32)
    W = {
        "wq": np.asarray(W_q, np.float32) / 8.0,
        "wk": np.asarray(W_k, np.float32),
        "wv": np.asarray(W_v, np.float32),
        "wu": np.asarray(W_u, np.float32),
    }
    Wo = np.asarray(W_o, np.float32)
    shared = {"ident": np.eye(128, dtype=BF), "masks": masks_np}
    for b in range(B):
        xb = x[b].astype(BF)  # [L, D]
        shared[f"x{b}"] = np.ascontiguousarray(
            xb.reshape(NIC, 512, NDC, 128).transpose(0, 3, 2, 1)
        )  # [c, p, dc, l']
        xkb = xb[sched[b]["perm"]].copy()  # [nk*128, D]
        xkb[sched[b]["nv"] :] = 0
        nch = len(sched[b]["kchunks"])
        xkp = np.zeros((nch * 512, D), BF)
        xkp[: xkb.shape[0]] = xkb
        shared[f"xk{b}"] = np.ascontiguousarray(
            xkp.reshape(nch, 512, NDC, 128).transpose(0, 3, 2, 1)
        )  # [ci, p, dc, j']
    in_maps = []
    for core in range(8):
        e0 = E * core
        im = dict(shared)
        for w, Wm in W.items():
            im[w] = np.ascontiguousarray(
                Wm[e0 : e0 + E].astype(BF).reshape(E, NDC, 128).transpose(2, 1, 0)
            )  # [p, dc, e]
        im["wo"] = np.ascontiguousarray(Wo[:, e0 : e0 + E].astype(BF).T)  # [p, d]
        in_maps.append(im)
    return in_maps


def kernel(x, token_types, seq_lens, W_q, W_k, W_v, W_u, W_o, **_run_kwargs):
    sig, sched, masks_np = _mk_schedule(np.asarray(token_types), np.asarray(seq_lens))
    if _cache.get("sig") != sig:
        _cache["nc"] = _build(sched, masks_np.shape[1])
        _cache["sig"] = sig
    nc = _cache["nc"]
    in_maps = _host_inputs(
        x, token_types, seq_lens, W_q, W_k, W_v, W_u, W_o, sched, masks_np
    )
    try:
        res = run_bass_kernel_spmd(nc, in_maps, list(range(8)), **_run_kwargs)
    except Exception as ex:
        if "UNRECOVERABLE" not in str(ex) and "UNAVAILABLE" not in str(ex):
            raise
        res = run_bass_kernel_spmd(nc, in_maps, list(range(8)), **_run_kwargs)
    _cache["last_result"] = res
    full = np.zeros((B, L, D), np.float64)
    for core in range(8):
        o = res.results[core]["outp"].astype(np.float64)  # [b, c, p, k, d]
        full += o.transpose(0, 1, 3, 2, 4).reshape(B, L, D)
    return full.astype(np.float32)
